# revision 6
# baseline (speedup 1.0000x reference)
"""Trainium2 Bass kernel for nn_BilinearAttentionFusion.

Math (see reference):
    b_mean = mean_j feat_b[b, j, :]                      [32, 512]
    t[b, k, d] = sum_e W[k, d, e] * b_mean[b, e]         [32, 512, 512]
    fused = feat_a @ t^T + bias                          [32, 300, 512]
    out = LayerNorm(fused + feat_a) * gamma + beta

Distribution (8 NeuronCores, 2 SPMD launches):
    L1 (k-sharded): core i owns W[64i:64(i+1)] (64 MB fp32).
        - reduces its 1/8 slice of feat_b (j-sharded) to a partial b_mean,
          AllReduce(add) across the 8 cores -> full b_meanT on every core
        - streams its W shard (host-transposed to [e, d, k_loc]) through the
          PE as the moving operand vs the tiny stationary b_meanT
          -> t_shard [32, (d, k_loc)] written to HBM
    host: concat t shards over k -> t[b, d, k], reshard by batch
    L2 (batch-sharded): core j owns batches 4j..4j+3.
        - fused[b, a, :] = feat_aT[b]^T @ t[b]  (contract d)
        - + bias + residual, LayerNorm over hidden, * gamma + beta

Matmuls run in float32r (full PE rate; ~1.5e-4 component rel-err).
All reductions/accumulations are fp32 (PSUM + DVE).
"""
import sys

for _p in ("/opt/trn_rl_repo", "/root/.axon_site", "/root/.axon_site/_ro/pypackages"):
    if _p not in sys.path:
        sys.path.append(_p)

import numpy as np
import concourse.bacc as bacc
import concourse.tile as tile
from concourse import mybir
from concourse.bass_utils import run_bass_kernel_spmd

N_CORES = 8
BS, LEN_A, LEN_B, H = 32, 300, 1024, 512
K_SH = H // N_CORES  # 64 k-columns of W per core in L1
B_SH = BS // N_CORES  # 4 batches per core in L2
LN_EPS = 1e-5

F32 = mybir.dt.float32
F32R = mybir.dt.float32r  # PE-native reduced fp32: 1 cyc/row vs 4 for fp32

# L1 W streaming tile: [128 e-partitions, WCOLS of (d, k_loc)] fp32 = 1 MiB
WCOLS = 2048
DK = H * K_SH  # 32768 flattened (d, k_loc) columns per core
N_GROUPS = DK // WCOLS  # 16
CHUNKS = WCOLS // 512  # 4 psum chunks per group
ET = H // 128  # 4 e-tiles (contraction)
A_TILES = [(0, 128), (128, 128), (256, 44)]  # len_a = 300


def _build_l1():
    nc = bacc.Bacc(trn_type="TRN2", num_devices=N_CORES)
    fbt = nc.dram_tensor("fbt", [H, BS, LEN_B // N_CORES], F32, kind="ExternalInput")
    wt = nc.dram_tensor("wt", [H, DK], F32R, kind="ExternalInput")
    t_out = nc.dram_tensor("t_out", [BS, DK], F32, kind="ExternalOutput")
    cc_in = nc.dram_tensor("cc_in", [H, BS], F32)
    cc_out = nc.dram_tensor("cc_out", [H, BS], F32, addr_space="Shared")

    with tile.TileContext(nc) as tc:
        with (
            tc.tile_pool(name="fb", bufs=2) as fbp,
            tc.tile_pool(name="bm", bufs=1) as bmp,
            tc.tile_pool(name="wtiles", bufs=14) as wp,
            tc.tile_pool(name="ps", bufs=4, space="PSUM") as ps,
            tc.tile_pool(name="tstage", bufs=2) as tsp,
            tc.tile_pool(name="small", bufs=4) as small,
        ):
            # ---- partial b_mean over this core's j-shard, scaled 1/len_b ----
            for et in range(ET):
                fb_t = fbp.tile([128, BS, LEN_B // N_CORES], F32)
                nc.sync.dma_start(out=fb_t[:], in_=fbt[et * 128 : (et + 1) * 128, :, :])
                pb = small.tile([128, BS], F32)
                nc.vector.reduce_sum(out=pb[:], in_=fb_t[:], axis=mybir.AxisListType.X)
                nc.scalar.mul(out=pb[:], in_=pb[:], mul=1.0 / LEN_B)
                nc.sync.dma_start(out=cc_in[et * 128 : (et + 1) * 128, :], in_=pb[:])

            nc.gpsimd.collective_compute(
                "AllReduce",
                mybir.AluOpType.add,
                ins=[cc_in.ap()],
                outs=[cc_out.ap()],
                replica_groups=[list(range(N_CORES))],
            )

            # full b_meanT, cast to fp32r for the PE (stationary operand)
            bmt = bmp.tile([128, ET, BS], F32R)
            nc.gpsimd.dma_start(
                out=bmt[:], in_=cc_out.ap().rearrange("(t p) b -> p t b", p=128)
            )

            # ---- stream W shard: t[b, dk] = sum_e b_meanT[e, b] * wt[e, dk] ----
            for g in range(N_GROUPS):
                wts = []
                for et in range(ET):
                    w_t = wp.tile([128, WCOLS], F32R, tag="wt")
                    nc.sync.dma_start(
                        out=w_t[:],
                        in_=wt[et * 128 : (et + 1) * 128, g * WCOLS : (g + 1) * WCOLS],
                    )
                    wts.append(w_t)
                stage = tsp.tile([BS, WCOLS], F32)
                for c in range(CHUNKS):
                    psum = ps.tile([BS, 512], F32)
                    for et in range(ET):
                        nc.tensor.matmul(
                            out=psum[:],
                            lhsT=bmt[:, et, :],
                            rhs=wts[et][:, c * 512 : (c + 1) * 512],
                            start=(et == 0),
                            stop=(et == ET - 1),
                        )
                    nc.vector.tensor_copy(stage[:, c * 512 : (c + 1) * 512], psum[:])
                nc.scalar.dma_start(
                    out=t_out[:, g * WCOLS : (g + 1) * WCOLS], in_=stage[:]
                )
    nc.finalize()
    return nc


def _build_l2():
    nc = bacc.Bacc(trn_type="TRN2", num_devices=N_CORES)
    tb = nc.dram_tensor("tb", [B_SH, H, H], F32R, kind="ExternalInput")  # [b, d, k]
    fatb = nc.dram_tensor("fatb", [B_SH, H, LEN_A], F32R, kind="ExternalInput")
    fab = nc.dram_tensor("fab", [B_SH, LEN_A, H], F32, kind="ExternalInput")
    bias_d = nc.dram_tensor("bias", [H], F32, kind="ExternalInput")
    gamma_d = nc.dram_tensor("gamma", [H], F32, kind="ExternalInput")
    beta_d = nc.dram_tensor("beta", [H], F32, kind="ExternalInput")
    out = nc.dram_tensor("out", [B_SH, LEN_A, H], F32, kind="ExternalOutput")

    with tile.TileContext(nc) as tc:
        with (
            tc.tile_pool(name="consts", bufs=1) as consts,
            tc.tile_pool(name="ins", bufs=3) as ins,
            tc.tile_pool(name="ps", bufs=4, space="PSUM") as ps,
            tc.tile_pool(name="work", bufs=4) as work,
            tc.tile_pool(name="small", bufs=8) as small,
        ):
            bias_t = consts.tile([128, H], F32)
            nc.sync.dma_start(out=bias_t[:], in_=bias_d.ap().partition_broadcast(128))
            gamma_t = consts.tile([128, H], F32)
            nc.sync.dma_start(out=gamma_t[:], in_=gamma_d.ap().partition_broadcast(128))
            beta_t = consts.tile([128, H], F32)
            nc.sync.dma_start(out=beta_t[:], in_=beta_d.ap().partition_broadcast(128))
            eps_t = consts.tile([128, 1], F32)
            nc.vector.memset(eps_t[:], LN_EPS)

            for b in range(B_SH):
                # t[b]: [512 d, 512 k] -> [128, 4 dt, 512]; feat_aT[b] likewise
                t_t = ins.tile([128, ET, H], F32R, tag="t")
                nc.sync.dma_start(
                    out=t_t[:], in_=tb[b].rearrange("(dt p) k -> p dt k", p=128)
                )
                fat_t = ins.tile([128, ET, LEN_A], F32R, tag="fat")
                nc.sync.dma_start(
                    out=fat_t[:], in_=fatb[b].rearrange("(dt p) a -> p dt a", p=128)
                )
                for a0, aw in A_TILES:
                    psum = ps.tile([aw, H], F32, tag="psum")
                    for dt_i in range(ET):
                        nc.tensor.matmul(
                            out=psum[:],
                            lhsT=fat_t[:, dt_i, a0 : a0 + aw],
                            rhs=t_t[:, dt_i, :],
                            start=(dt_i == 0),
                            stop=(dt_i == ET - 1),
                        )
                    fa_t = work.tile([aw, H], F32, tag="fa")
                    nc.sync.dma_start(out=fa_t[:], in_=fab[b, a0 : a0 + aw, :])
                    x = work.tile([aw, H], F32, tag="x")
                    nc.vector.tensor_add(out=x[:], in0=psum[:], in1=fa_t[:])
                    nc.vector.tensor_add(out=x[:], in0=x[:], in1=bias_t[:aw, :])
                    stats = small.tile([aw, 6], F32, tag="stats")
                    nc.vector.bn_stats(out=stats[:], in_=x[:])
                    mv = small.tile([aw, 2], F32, tag="mv")
                    nc.vector.bn_aggr(out=mv[:], in_=stats[:])
                    rstd = small.tile([aw, 1], F32, tag="rstd")
                    nc.scalar.activation(
                        out=rstd[:],
                        in_=mv[:, 1:2],
                        func=mybir.ActivationFunctionType.Sqrt,
                        bias=eps_t[:aw, :],
                        scale=1.0,
                    )
                    nc.vector.reciprocal(out=rstd[:], in_=rstd[:])
                    xn = work.tile([aw, H], F32, tag="xn")
                    nc.vector.tensor_scalar(
                        out=xn[:],
                        in0=x[:],
                        scalar1=mv[:, 0:1],
                        scalar2=rstd[:],
                        op0=mybir.AluOpType.subtract,
                        op1=mybir.AluOpType.mult,
                    )
                    nc.vector.tensor_mul(out=xn[:], in0=xn[:], in1=gamma_t[:aw, :])
                    nc.vector.tensor_add(out=xn[:], in0=xn[:], in1=beta_t[:aw, :])
                    nc.scalar.dma_start(out=out[b, a0 : a0 + aw, :], in_=xn[:])
    nc.finalize()
    return nc


_NC_L1 = None
_NC_L2 = None


def _programs():
    global _NC_L1, _NC_L2
    if _NC_L1 is None:
        _NC_L1 = _build_l1()
        _NC_L2 = _build_l2()
    return _NC_L1, _NC_L2


def kernel(feat_a, feat_b, W, bias, gamma, beta, _trace=False, _timings=None):
    feat_a = np.ascontiguousarray(feat_a, dtype=np.float32)
    feat_b = np.ascontiguousarray(feat_b, dtype=np.float32)
    W = np.ascontiguousarray(W, dtype=np.float32)
    bias = np.ascontiguousarray(bias, dtype=np.float32)
    gamma = np.ascontiguousarray(gamma, dtype=np.float32)
    beta = np.ascontiguousarray(beta, dtype=np.float32)

    nc1, nc2 = _programs()
    core_ids = list(range(N_CORES))
    trace_kw = dict(trace=True, trace_cores=[0]) if _trace else {}

    # ---- L1: host-side shard layout ----
    jb = LEN_B // N_CORES
    in_maps1 = []
    for i in range(N_CORES):
        # feat_b j-shard, transposed to [e, b, j]
        fbt_i = np.ascontiguousarray(
            feat_b[:, i * jb : (i + 1) * jb, :].transpose(2, 0, 1)
        )
        # W k-shard [64, 512, 512] -> [e, d, k_loc] -> flat [512, 32768]
        wi = np.ascontiguousarray(
            W[i * K_SH : (i + 1) * K_SH].transpose(2, 1, 0)
        ).reshape(H, DK)
        in_maps1.append({"fbt": fbt_i, "wt": wi})
    res1 = run_bass_kernel_spmd(nc1, in_maps1, core_ids, **trace_kw)
    if _timings is not None:
        _timings.append(res1.exec_time_ns)

    # t shards [32, (d, k_loc)] -> full t [b, d, k]
    t_full = np.concatenate(
        [res1.results[i]["t_out"].reshape(BS, H, K_SH) for i in range(N_CORES)], axis=2
    )

    # ---- L2: batch shards ----
    in_maps2 = []
    for j in range(N_CORES):
        bs = slice(j * B_SH, (j + 1) * B_SH)
        in_maps2.append(
            {
                "tb": np.ascontiguousarray(t_full[bs]),
                "fatb": np.ascontiguousarray(feat_a[bs].transpose(0, 2, 1)),
                "fab": np.ascontiguousarray(feat_a[bs]),
                "bias": bias,
                "gamma": gamma,
                "beta": beta,
            }
        )
    res2 = run_bass_kernel_spmd(nc2, in_maps2, core_ids, **trace_kw)
    if _timings is not None:
        _timings.append(res2.exec_time_ns)

    return np.concatenate([res2.results[j]["out"] for j in range(N_CORES)], axis=0)


# revision 9
# speedup vs baseline: 1.1671x; 1.1671x over previous
"""Trainium2 Bass kernel for nn_BilinearAttentionFusion.

Math (see reference):
    b_mean = mean_j feat_b[b, j, :]                      [32, 512]
    t[b, k, d] = sum_e W[k, d, e] * b_mean[b, e]         [32, 512, 512]
    fused = feat_a @ t^T + bias                          [32, 300, 512]
    out = LayerNorm(fused + feat_a) * gamma + beta

Distribution (8 NeuronCores, 3 SPMD launches, no collectives —
collectives cost 60-170us of cross-core sync under this runtime):
    K1 (j-sharded): core i reduces feat_b[:, 128i:128(i+1), :] to a
        partial b_meanT [e, b] (scaled 1/1024). Host sums the 8 partials.
    K2 (k-sharded): core i owns W[64i:64(i+1)] (64 MB fp32),
        host-transposed to [e, (d, k_loc)]. Streams it through the PE as
        the moving operand vs the tiny stationary b_meanT
        -> t_shard [32, (d, k_loc)]. Pure per-core streaming, no sync.
    host: concat t shards over k -> t[b, d, k], reshard by batch.
    K3 (batch-sharded): core j owns batches 4j..4j+3:
        fused[b] = feat_aT[b]^T @ t[b] (contract d), bias folded into the
        matmul as a K=1 accumulation row, + residual, LayerNorm, gamma/beta.

Matmuls run in float32r (full PE rate; ~1.5e-4 component rel-err).
All reductions/accumulations are fp32 (PSUM + DVE).
"""
import sys

for _p in ("/opt/trn_rl_repo", "/root/.axon_site", "/root/.axon_site/_ro/pypackages"):
    if _p not in sys.path:
        sys.path.append(_p)

import numpy as np
import concourse.bacc as bacc
import concourse.tile as tile
from concourse import mybir
from concourse.bass_utils import run_bass_kernel_spmd

N_CORES = 8
BS, LEN_A, LEN_B, H = 32, 300, 1024, 512
K_SH = H // N_CORES  # 64 k-columns of W per core in K2
B_SH = BS // N_CORES  # 4 batches per core in K3
J_SH = LEN_B // N_CORES  # 128 j-rows of feat_b per core in K1
LN_EPS = 1e-5

F32 = mybir.dt.float32
F32R = mybir.dt.float32r  # PE-native reduced fp32: full-rate stream vs 4 cyc/row

WCOLS = 2048  # K2 W-streaming tile free size (1 MiB tiles)
DK = H * K_SH  # 32768 flattened (d, k_loc) columns per core
N_GROUPS = DK // WCOLS  # 16
CHUNKS = WCOLS // 512  # 4 psum chunks per group
ET = H // 128  # 4 contraction e-tiles
A_TILES = [(0, 128), (128, 128), (256, 44)]  # len_a = 300


def _build_k1():
    nc = bacc.Bacc(trn_type="TRN2", num_devices=N_CORES)
    fbt = nc.dram_tensor("fbt", [H, BS, J_SH], F32, kind="ExternalInput")
    pb_out = nc.dram_tensor("pb", [H, BS], F32, kind="ExternalOutput")
    with tile.TileContext(nc) as tc:
        with (
            tc.tile_pool(name="fb", bufs=2) as fbp,
            tc.tile_pool(name="small", bufs=4) as small,
        ):
            for et in range(ET):
                fb_t = fbp.tile([128, BS, J_SH], F32)
                nc.sync.dma_start(out=fb_t[:], in_=fbt[et * 128 : (et + 1) * 128, :, :])
                pb = small.tile([128, BS], F32)
                nc.vector.reduce_sum(out=pb[:], in_=fb_t[:], axis=mybir.AxisListType.X)
                nc.scalar.mul(out=pb[:], in_=pb[:], mul=1.0 / LEN_B)
                nc.scalar.dma_start(out=pb_out[et * 128 : (et + 1) * 128, :], in_=pb[:])
    nc.finalize()
    return nc


def _build_k2():
    nc = bacc.Bacc(trn_type="TRN2", num_devices=N_CORES)
    bm = nc.dram_tensor("bm", [H, BS], F32R, kind="ExternalInput")
    wt = nc.dram_tensor("wt", [H, DK], F32R, kind="ExternalInput")
    t_out = nc.dram_tensor("t_out", [BS, DK], F32, kind="ExternalOutput")

    with tile.TileContext(nc) as tc:
        with (
            tc.tile_pool(name="bm", bufs=1) as bmp,
            tc.tile_pool(name="wtiles", bufs=12) as wp,
            tc.tile_pool(name="ps", bufs=8, space="PSUM") as ps,
            tc.tile_pool(name="tstage", bufs=3) as tsp,
        ):
            bmt = bmp.tile([128, ET, BS], F32R)
            nc.sync.dma_start(out=bmt[:], in_=bm.ap().rearrange("(t p) b -> p t b", p=128))

            for g in range(N_GROUPS):
                wts = []
                for et in range(ET):
                    w_t = wp.tile([128, WCOLS], F32R, tag="wt")
                    nc.sync.dma_start(
                        out=w_t[:],
                        in_=wt[et * 128 : (et + 1) * 128, g * WCOLS : (g + 1) * WCOLS],
                    )
                    wts.append(w_t)
                psums = [
                    ps.tile([BS, 512], F32, tag="psum", name=f"psum{c}")
                    for c in range(CHUNKS)
                ]
                # et-outer: stationary b_meanT[et] reused across CHUNKS matmuls
                for et in range(ET):
                    for c in range(CHUNKS):
                        nc.tensor.matmul(
                            out=psums[c][:],
                            lhsT=bmt[:, et, :],
                            rhs=wts[et][:, c * 512 : (c + 1) * 512],
                            start=(et == 0),
                            stop=(et == ET - 1),
                        )
                stage = tsp.tile([BS, WCOLS], F32)
                for c in range(CHUNKS):
                    nc.vector.tensor_copy(stage[:, c * 512 : (c + 1) * 512], psums[c][:])
                nc.scalar.dma_start(
                    out=t_out[:, g * WCOLS : (g + 1) * WCOLS], in_=stage[:]
                )
    nc.finalize()
    return nc


def _build_k3():
    nc = bacc.Bacc(trn_type="TRN2", num_devices=N_CORES)
    tb = nc.dram_tensor("tb", [B_SH, H, H], F32R, kind="ExternalInput")  # [b, d, k]
    fatb = nc.dram_tensor("fatb", [B_SH, H, LEN_A], F32R, kind="ExternalInput")
    fab = nc.dram_tensor("fab", [B_SH, LEN_A, H], F32, kind="ExternalInput")
    bias_d = nc.dram_tensor("bias", [H], F32R, kind="ExternalInput")
    gamma_d = nc.dram_tensor("gamma", [H], F32, kind="ExternalInput")
    beta_d = nc.dram_tensor("beta", [H], F32, kind="ExternalInput")
    out = nc.dram_tensor("out", [B_SH, LEN_A, H], F32, kind="ExternalOutput")

    with tile.TileContext(nc) as tc:
        with (
            tc.tile_pool(name="consts", bufs=1) as consts,
            tc.tile_pool(name="ins", bufs=3) as ins,
            tc.tile_pool(name="ps", bufs=4, space="PSUM") as ps,
            tc.tile_pool(name="work", bufs=4) as work,
            tc.tile_pool(name="small", bufs=8) as small,
        ):
            gamma_t = consts.tile([128, H], F32)
            nc.sync.dma_start(out=gamma_t[:], in_=gamma_d.ap().partition_broadcast(128))
            beta_t = consts.tile([128, H], F32)
            nc.sync.dma_start(out=beta_t[:], in_=beta_d.ap().partition_broadcast(128))
            eps_t = consts.tile([128, 1], F32)
            nc.vector.memset(eps_t[:], LN_EPS)
            # bias folded into the matmul: ones[1, aw] (lhsT) x bias[1, 512] (rhs)
            bias_row = consts.tile([1, H], F32R)
            nc.sync.dma_start(out=bias_row[:], in_=bias_d.ap()[None, :])
            ones_f = consts.tile([1, 128], F32)
            nc.vector.memset(ones_f[:], 1.0)
            ones_row = consts.tile([1, 128], F32R)
            nc.vector.tensor_copy(ones_row[:], ones_f[:])

            for b in range(B_SH):
                t_t = ins.tile([128, ET, H], F32R, tag="t")
                nc.sync.dma_start(
                    out=t_t[:], in_=tb[b].rearrange("(dt p) k -> p dt k", p=128)
                )
                fat_t = ins.tile([128, ET, LEN_A], F32R, tag="fat")
                nc.sync.dma_start(
                    out=fat_t[:], in_=fatb[b].rearrange("(dt p) a -> p dt a", p=128)
                )
                for a0, aw in A_TILES:
                    psum = ps.tile([aw, H], F32, tag="psum")
                    nc.tensor.matmul(
                        out=psum[:],
                        lhsT=ones_row[:, :aw],
                        rhs=bias_row[:],
                        start=True,
                        stop=False,
                    )
                    for dt_i in range(ET):
                        nc.tensor.matmul(
                            out=psum[:],
                            lhsT=fat_t[:, dt_i, a0 : a0 + aw],
                            rhs=t_t[:, dt_i, :],
                            start=False,
                            stop=(dt_i == ET - 1),
                        )
                    fa_t = work.tile([aw, H], F32, tag="fa")
                    nc.sync.dma_start(out=fa_t[:], in_=fab[b, a0 : a0 + aw, :])
                    x = work.tile([aw, H], F32, tag="x")
                    nc.vector.tensor_add(out=x[:], in0=psum[:], in1=fa_t[:])
                    stats = small.tile([aw, 6], F32, tag="stats")
                    nc.vector.bn_stats(out=stats[:], in_=x[:])
                    mv = small.tile([aw, 2], F32, tag="mv")
                    nc.vector.bn_aggr(out=mv[:], in_=stats[:])
                    rstd = small.tile([aw, 1], F32, tag="rstd")
                    nc.scalar.activation(
                        out=rstd[:],
                        in_=mv[:, 1:2],
                        func=mybir.ActivationFunctionType.Sqrt,
                        bias=eps_t[:aw, :],
                        scale=1.0,
                    )
                    nc.vector.reciprocal(out=rstd[:], in_=rstd[:])
                    xn = work.tile([aw, H], F32, tag="xn")
                    nc.vector.tensor_scalar(
                        out=xn[:],
                        in0=x[:],
                        scalar1=mv[:, 0:1],
                        scalar2=rstd[:],
                        op0=mybir.AluOpType.subtract,
                        op1=mybir.AluOpType.mult,
                    )
                    nc.vector.tensor_mul(out=xn[:], in0=xn[:], in1=gamma_t[:aw, :])
                    nc.vector.tensor_add(out=xn[:], in0=xn[:], in1=beta_t[:aw, :])
                    nc.scalar.dma_start(out=out[b, a0 : a0 + aw, :], in_=xn[:])
    nc.finalize()
    return nc


_NCS = None


def _programs():
    global _NCS
    if _NCS is None:
        _NCS = (_build_k1(), _build_k2(), _build_k3())
    return _NCS


def kernel(feat_a, feat_b, W, bias, gamma, beta, _trace=False, _timings=None):
    feat_a = np.ascontiguousarray(feat_a, dtype=np.float32)
    feat_b = np.ascontiguousarray(feat_b, dtype=np.float32)
    W = np.ascontiguousarray(W, dtype=np.float32)
    bias = np.ascontiguousarray(bias, dtype=np.float32)
    gamma = np.ascontiguousarray(gamma, dtype=np.float32)
    beta = np.ascontiguousarray(beta, dtype=np.float32)

    nc1, nc2, nc3 = _programs()
    core_ids = list(range(N_CORES))
    trace_kw = dict(trace=True, trace_cores=[0]) if _trace else {}

    # ---- K1: partial b_mean over j-shards ----
    in_maps1 = [
        {
            "fbt": np.ascontiguousarray(
                feat_b[:, i * J_SH : (i + 1) * J_SH, :].transpose(2, 0, 1)
            )
        }
        for i in range(N_CORES)
    ]
    res1 = run_bass_kernel_spmd(nc1, in_maps1, core_ids, **trace_kw)
    if _timings is not None:
        _timings.append(res1.exec_time_ns)
    bmT = np.sum([res1.results[i]["pb"] for i in range(N_CORES)], axis=0)
    bmT = np.ascontiguousarray(bmT, dtype=np.float32)

    # ---- K2: t = W x b_mean, k-sharded W stream ----
    in_maps2 = []
    for i in range(N_CORES):
        wi = np.ascontiguousarray(
            W[i * K_SH : (i + 1) * K_SH].transpose(2, 1, 0)
        ).reshape(H, DK)
        in_maps2.append({"bm": bmT, "wt": wi})
    res2 = run_bass_kernel_spmd(nc2, in_maps2, core_ids, **trace_kw)
    if _timings is not None:
        _timings.append(res2.exec_time_ns)
    t_full = np.concatenate(
        [res2.results[i]["t_out"].reshape(BS, H, K_SH) for i in range(N_CORES)], axis=2
    )

    # ---- K3: fused matmul + residual + LayerNorm, batch-sharded ----
    in_maps3 = []
    for j in range(N_CORES):
        bsl = slice(j * B_SH, (j + 1) * B_SH)
        in_maps3.append(
            {
                "tb": np.ascontiguousarray(t_full[bsl]),
                "fatb": np.ascontiguousarray(feat_a[bsl].transpose(0, 2, 1)),
                "fab": np.ascontiguousarray(feat_a[bsl]),
                "bias": bias,
                "gamma": gamma,
                "beta": beta,
            }
        )
    res3 = run_bass_kernel_spmd(nc3, in_maps3, core_ids, **trace_kw)
    if _timings is not None:
        _timings.append(res3.exec_time_ns)

    return np.concatenate([res3.results[j]["out"] for j in range(N_CORES)], axis=0)


# revision 11
# speedup vs baseline: 1.1805x; 1.0115x over previous
"""Trainium2 Bass kernel for nn_BilinearAttentionFusion.

Math (see reference):
    b_mean = mean_j feat_b[b, j, :]                      [32, 512]
    t[b, k, d] = sum_e W[k, d, e] * b_mean[b, e]         [32, 512, 512]
    fused = feat_a @ t^T + bias                          [32, 300, 512]
    out = LayerNorm(fused + feat_a) * gamma + beta

Distribution (8 NeuronCores, 3 SPMD launches, no collectives —
collectives cost 60-170us of cross-core sync under this runtime):
    K1 (j-sharded): core i reduces feat_b[:, 128i:128(i+1), :] to a
        partial b_meanT [e, b] (scaled 1/1024). Host sums the 8 partials.
    K2 (k-sharded): core i owns W[64i:64(i+1)] (64 MB fp32),
        host-transposed to [e, (d, k_loc)]. Streams it through the PE as
        the moving operand vs the tiny stationary b_meanT
        -> t_shard [32, (d, k_loc)]. Pure per-core streaming, no sync.
    host: concat t shards over k -> t[b, d, k], reshard by batch.
    K3 (batch-sharded): core j owns batches 4j..4j+3:
        fused[b] = feat_aT[b]^T @ t[b] (contract d), + bias + residual,
        LayerNorm, gamma/beta (skipped when exactly ones/zeros).

All matmuls, reductions and accumulations are full fp32 (the fp32 PE
moving-operand cost of 4 cyc/row overlaps the HBM-bound W stream).
"""
import sys

for _p in ("/opt/trn_rl_repo", "/root/.axon_site", "/root/.axon_site/_ro/pypackages"):
    if _p not in sys.path:
        sys.path.append(_p)

import numpy as np
import concourse.bacc as bacc
import concourse.tile as tile
from concourse import mybir
from concourse.bass_utils import run_bass_kernel_spmd

N_CORES = 8
BS, LEN_A, LEN_B, H = 32, 300, 1024, 512
K_SH = H // N_CORES  # 64 k-columns of W per core in K2
B_SH = BS // N_CORES  # 4 batches per core in K3
J_SH = LEN_B // N_CORES  # 128 j-rows of feat_b per core in K1
LN_EPS = 1e-5

F32 = mybir.dt.float32
F32R = mybir.dt.float32r  # PE-native reduced fp32: full-rate stream vs 4 cyc/row

WCOLS = 2048  # K2 W-streaming tile free size (1 MiB tiles)
DK = H * K_SH  # 32768 flattened (d, k_loc) columns per core
N_GROUPS = DK // WCOLS  # 16
CHUNKS = WCOLS // 512  # 4 psum chunks per group
ET = H // 128  # 4 contraction e-tiles
A_TILES = [(0, 128), (128, 128), (256, 44)]  # len_a = 300


def _build_k1():
    nc = bacc.Bacc(trn_type="TRN2", num_devices=N_CORES)
    fbt = nc.dram_tensor("fbt", [H, BS, J_SH], F32, kind="ExternalInput")
    pb_out = nc.dram_tensor("pb", [H, BS], F32, kind="ExternalOutput")
    with tile.TileContext(nc) as tc:
        with (
            tc.tile_pool(name="fb", bufs=3) as fbp,
            tc.tile_pool(name="small", bufs=4) as small,
        ):
            for et in range(ET):
                fb_t = fbp.tile([128, BS, J_SH], F32)
                nc.sync.dma_start(out=fb_t[:], in_=fbt[et * 128 : (et + 1) * 128, :, :])
                pb = small.tile([128, BS], F32)
                nc.vector.reduce_sum(out=pb[:], in_=fb_t[:], axis=mybir.AxisListType.X)
                nc.scalar.mul(out=pb[:], in_=pb[:], mul=1.0 / LEN_B)
                nc.scalar.dma_start(out=pb_out[et * 128 : (et + 1) * 128, :], in_=pb[:])
    nc.finalize()
    return nc


def _build_k2():
    nc = bacc.Bacc(trn_type="TRN2", num_devices=N_CORES)
    bm = nc.dram_tensor("bm", [H, BS], F32, kind="ExternalInput")
    wt = nc.dram_tensor("wt", [H, DK], F32, kind="ExternalInput")
    t_out = nc.dram_tensor("t_out", [BS, DK], F32, kind="ExternalOutput")

    with tile.TileContext(nc) as tc:
        with (
            tc.tile_pool(name="bm", bufs=1) as bmp,
            tc.tile_pool(name="wtiles", bufs=12) as wp,
            tc.tile_pool(name="ps", bufs=8, space="PSUM") as ps,
            tc.tile_pool(name="tstage", bufs=3) as tsp,
        ):
            bmt = bmp.tile([128, ET, BS], F32)
            nc.sync.dma_start(out=bmt[:], in_=bm.ap().rearrange("(t p) b -> p t b", p=128))

            for g in range(N_GROUPS):
                wts = []
                for et in range(ET):
                    w_t = wp.tile([128, WCOLS], F32, tag="wt")
                    nc.sync.dma_start(
                        out=w_t[:],
                        in_=wt[et * 128 : (et + 1) * 128, g * WCOLS : (g + 1) * WCOLS],
                    )
                    wts.append(w_t)
                psums = [
                    ps.tile([BS, 512], F32, tag="psum", name=f"psum{c}")
                    for c in range(CHUNKS)
                ]
                # et-outer: stationary b_meanT[et] reused across CHUNKS matmuls
                for et in range(ET):
                    for c in range(CHUNKS):
                        nc.tensor.matmul(
                            out=psums[c][:],
                            lhsT=bmt[:, et, :],
                            rhs=wts[et][:, c * 512 : (c + 1) * 512],
                            start=(et == 0),
                            stop=(et == ET - 1),
                        )
                stage = tsp.tile([BS, WCOLS], F32)
                for c in range(CHUNKS):
                    nc.vector.tensor_copy(stage[:, c * 512 : (c + 1) * 512], psums[c][:])
                nc.scalar.dma_start(
                    out=t_out[:, g * WCOLS : (g + 1) * WCOLS], in_=stage[:]
                )
    nc.finalize()
    return nc


def _build_k3(apply_affine):
    nc = bacc.Bacc(trn_type="TRN2", num_devices=N_CORES)
    tb = nc.dram_tensor("tb", [B_SH, H, H], F32, kind="ExternalInput")  # [b, d, k]
    fatb = nc.dram_tensor("fatb", [B_SH, H, LEN_A], F32, kind="ExternalInput")
    fab = nc.dram_tensor("fab", [B_SH, LEN_A, H], F32, kind="ExternalInput")
    bias_d = nc.dram_tensor("bias", [H], F32, kind="ExternalInput")
    gamma_d = nc.dram_tensor("gamma", [H], F32, kind="ExternalInput")
    beta_d = nc.dram_tensor("beta", [H], F32, kind="ExternalInput")
    out = nc.dram_tensor("out", [B_SH, LEN_A, H], F32, kind="ExternalOutput")

    with tile.TileContext(nc) as tc:
        with (
            tc.tile_pool(name="consts", bufs=1) as consts,
            tc.tile_pool(name="ins", bufs=3) as ins,
            tc.tile_pool(name="ps", bufs=4, space="PSUM") as ps,
            tc.tile_pool(name="work", bufs=4) as work,
            tc.tile_pool(name="small", bufs=8) as small,
        ):
            gamma_t = consts.tile([128, H], F32)
            nc.sync.dma_start(out=gamma_t[:], in_=gamma_d.ap().partition_broadcast(128))
            beta_t = consts.tile([128, H], F32)
            nc.sync.dma_start(out=beta_t[:], in_=beta_d.ap().partition_broadcast(128))
            eps_t = consts.tile([128, 1], F32)
            nc.vector.memset(eps_t[:], LN_EPS)
            bias_t = consts.tile([128, H], F32)
            nc.sync.dma_start(out=bias_t[:], in_=bias_d.ap().partition_broadcast(128))

            for b in range(B_SH):
                t_t = ins.tile([128, ET, H], F32, tag="t")
                nc.sync.dma_start(
                    out=t_t[:], in_=tb[b].rearrange("(dt p) k -> p dt k", p=128)
                )
                fat_t = ins.tile([128, ET, LEN_A], F32, tag="fat")
                nc.sync.dma_start(
                    out=fat_t[:], in_=fatb[b].rearrange("(dt p) a -> p dt a", p=128)
                )
                for a0, aw in A_TILES:
                    psum = ps.tile([aw, H], F32, tag="psum")
                    for dt_i in range(ET):
                        nc.tensor.matmul(
                            out=psum[:],
                            lhsT=fat_t[:, dt_i, a0 : a0 + aw],
                            rhs=t_t[:, dt_i, :],
                            start=(dt_i == 0),
                            stop=(dt_i == ET - 1),
                        )
                    fa_t = work.tile([aw, H], F32, tag="fa")
                    nc.sync.dma_start(out=fa_t[:], in_=fab[b, a0 : a0 + aw, :])
                    x = work.tile([aw, H], F32, tag="x")
                    nc.vector.tensor_add(out=x[:], in0=psum[:], in1=fa_t[:])
                    nc.vector.tensor_add(out=x[:], in0=x[:], in1=bias_t[:aw, :])
                    stats = small.tile([aw, 6], F32, tag="stats")
                    nc.vector.bn_stats(out=stats[:], in_=x[:])
                    mv = small.tile([aw, 2], F32, tag="mv")
                    nc.vector.bn_aggr(out=mv[:], in_=stats[:])
                    rstd = small.tile([aw, 1], F32, tag="rstd")
                    nc.scalar.activation(
                        out=rstd[:],
                        in_=mv[:, 1:2],
                        func=mybir.ActivationFunctionType.Sqrt,
                        bias=eps_t[:aw, :],
                        scale=1.0,
                    )
                    nc.vector.reciprocal(out=rstd[:], in_=rstd[:])
                    xn = work.tile([aw, H], F32, tag="xn")
                    nc.vector.tensor_scalar(
                        out=xn[:],
                        in0=x[:],
                        scalar1=mv[:, 0:1],
                        scalar2=rstd[:],
                        op0=mybir.AluOpType.subtract,
                        op1=mybir.AluOpType.mult,
                    )
                    if apply_affine:
                        nc.vector.tensor_mul(out=xn[:], in0=xn[:], in1=gamma_t[:aw, :])
                        nc.vector.tensor_add(out=xn[:], in0=xn[:], in1=beta_t[:aw, :])
                    nc.scalar.dma_start(out=out[b, a0 : a0 + aw, :], in_=xn[:])
    nc.finalize()
    return nc


_CACHE = {}


def _program(name, builder):
    if name not in _CACHE:
        _CACHE[name] = builder()
    return _CACHE[name]


def kernel(feat_a, feat_b, W, bias, gamma, beta, _trace=False, _timings=None):
    feat_a = np.ascontiguousarray(feat_a, dtype=np.float32)
    feat_b = np.ascontiguousarray(feat_b, dtype=np.float32)
    W = np.ascontiguousarray(W, dtype=np.float32)
    bias = np.ascontiguousarray(bias, dtype=np.float32)
    gamma = np.ascontiguousarray(gamma, dtype=np.float32)
    beta = np.ascontiguousarray(beta, dtype=np.float32)

    core_ids = list(range(N_CORES))
    affine = not (np.all(gamma == 1.0) and np.all(beta == 0.0))
    nc1 = _program("k1", _build_k1)
    nc2 = _program("k2", _build_k2)
    nc3 = _program(("k3", affine), lambda: _build_k3(affine))
    trace_kw = dict(trace=True, trace_cores=[0]) if _trace else {}

    # ---- K1: partial b_mean over j-shards ----
    in_maps1 = [
        {
            "fbt": np.ascontiguousarray(
                feat_b[:, i * J_SH : (i + 1) * J_SH, :].transpose(2, 0, 1)
            )
        }
        for i in range(N_CORES)
    ]
    res1 = run_bass_kernel_spmd(nc1, in_maps1, core_ids, **trace_kw)
    if _timings is not None:
        _timings.append(res1.exec_time_ns)
    bmT = np.sum([res1.results[i]["pb"] for i in range(N_CORES)], axis=0)
    bmT = np.ascontiguousarray(bmT, dtype=np.float32)

    # ---- K2: t = W x b_mean, k-sharded W stream ----
    in_maps2 = []
    for i in range(N_CORES):
        wi = np.ascontiguousarray(
            W[i * K_SH : (i + 1) * K_SH].transpose(2, 1, 0)
        ).reshape(H, DK)
        in_maps2.append({"bm": bmT, "wt": wi})
    res2 = run_bass_kernel_spmd(nc2, in_maps2, core_ids, **trace_kw)
    if _timings is not None:
        _timings.append(res2.exec_time_ns)
    t_full = np.concatenate(
        [res2.results[i]["t_out"].reshape(BS, H, K_SH) for i in range(N_CORES)], axis=2
    )

    # ---- K3: fused matmul + residual + LayerNorm, batch-sharded ----
    in_maps3 = []
    for j in range(N_CORES):
        bsl = slice(j * B_SH, (j + 1) * B_SH)
        in_maps3.append(
            {
                "tb": np.ascontiguousarray(t_full[bsl]),
                "fatb": np.ascontiguousarray(feat_a[bsl].transpose(0, 2, 1)),
                "fab": np.ascontiguousarray(feat_a[bsl]),
                "bias": bias,
                "gamma": gamma,
                "beta": beta,
            }
        )
    res3 = run_bass_kernel_spmd(nc3, in_maps3, core_ids, **trace_kw)
    if _timings is not None:
        _timings.append(res3.exec_time_ns)

    return np.concatenate([res3.results[j]["out"] for j in range(N_CORES)], axis=0)


# revision 12
# speedup vs baseline: 1.2142x; 1.0285x over previous
"""Trainium2 Bass kernel for nn_BilinearAttentionFusion.

Math (see reference):
    b_mean = mean_j feat_b[b, j, :]                      [32, 512]
    t[b, k, d] = sum_e W[k, d, e] * b_mean[b, e]         [32, 512, 512]
    fused = feat_a @ t^T + bias                          [32, 300, 512]
    out = LayerNorm(fused + feat_a) * gamma + beta

Distribution (8 NeuronCores, 3 SPMD launches, no collectives —
collectives cost 60-170us of cross-core sync under this runtime):
    K1 (j-sharded): core i reduces feat_b[:, 128i:128(i+1), :] to a
        partial b_meanT [e, b] (scaled 1/1024). Host sums the 8 partials.
    K2 (k-sharded): core i owns W[64i:64(i+1)] (64 MB fp32),
        host-transposed to [e, (d, k_loc)]. Streams it through the PE as
        the moving operand vs the tiny stationary b_meanT
        -> t_shard [32, (d, k_loc)]. Pure per-core streaming, no sync.
    host: concat t shards over k -> t[b, d, k], reshard by batch.
    K3 (batch-sharded): core j owns batches 4j..4j+3:
        fused[b] = feat_aT[b]^T @ t[b] (contract d), + bias + residual,
        LayerNorm, gamma/beta (skipped when exactly ones/zeros).

All matmuls, reductions and accumulations are full fp32 (the fp32 PE
moving-operand cost of 4 cyc/row overlaps the HBM-bound W stream).
"""
import sys

for _p in ("/opt/trn_rl_repo", "/root/.axon_site", "/root/.axon_site/_ro/pypackages"):
    if _p not in sys.path:
        sys.path.append(_p)

import numpy as np
import concourse.bacc as bacc
import concourse.tile as tile
from concourse import mybir
from concourse.bass_utils import run_bass_kernel_spmd

N_CORES = 8
BS, LEN_A, LEN_B, H = 32, 300, 1024, 512
K_SH = H // N_CORES  # 64 k-columns of W per core in K2
B_SH = BS // N_CORES  # 4 batches per core in K3
J_SH = LEN_B // N_CORES  # 128 j-rows of feat_b per core in K1
LN_EPS = 1e-5

F32 = mybir.dt.float32
F32R = mybir.dt.float32r  # PE-native reduced fp32: full-rate stream vs 4 cyc/row

WCOLS = 2048  # K2 W-streaming tile free size (1 MiB tiles)
DK = H * K_SH  # 32768 flattened (d, k_loc) columns per core
N_GROUPS = DK // WCOLS  # 16
CHUNKS = WCOLS // 512  # 4 psum chunks per group
ET = H // 128  # 4 contraction e-tiles
A_TILES = [(0, 128), (128, 128), (256, 44)]  # len_a = 300


def _build_k1():
    nc = bacc.Bacc(trn_type="TRN2", num_devices=N_CORES)
    fbt = nc.dram_tensor("fbt", [H, BS, J_SH], F32, kind="ExternalInput")
    pb_out = nc.dram_tensor("pb", [H, BS], F32, kind="ExternalOutput")
    with tile.TileContext(nc) as tc:
        with (
            tc.tile_pool(name="fb", bufs=3) as fbp,
            tc.tile_pool(name="small", bufs=4) as small,
        ):
            for et in range(ET):
                fb_t = fbp.tile([128, BS, J_SH], F32)
                nc.sync.dma_start(out=fb_t[:], in_=fbt[et * 128 : (et + 1) * 128, :, :])
                pb = small.tile([128, BS], F32)
                nc.vector.reduce_sum(out=pb[:], in_=fb_t[:], axis=mybir.AxisListType.X)
                nc.scalar.mul(out=pb[:], in_=pb[:], mul=1.0 / LEN_B)
                nc.scalar.dma_start(out=pb_out[et * 128 : (et + 1) * 128, :], in_=pb[:])
    nc.finalize()
    return nc


def _build_k2():
    nc = bacc.Bacc(trn_type="TRN2", num_devices=N_CORES)
    bm = nc.dram_tensor("bm", [H, BS], F32, kind="ExternalInput")
    wt = nc.dram_tensor("wt", [H, DK], F32, kind="ExternalInput")
    t_out = nc.dram_tensor("t_out", [BS, DK], F32, kind="ExternalOutput")

    with tile.TileContext(nc) as tc:
        with (
            tc.tile_pool(name="bm", bufs=1) as bmp,
            tc.tile_pool(name="wtiles", bufs=12) as wp,
            tc.tile_pool(name="ps", bufs=8, space="PSUM") as ps,
            tc.tile_pool(name="tstage", bufs=3) as tsp,
        ):
            bmt = bmp.tile([128, ET, BS], F32)
            nc.sync.dma_start(out=bmt[:], in_=bm.ap().rearrange("(t p) b -> p t b", p=128))

            for g in range(N_GROUPS):
                wts = []
                for et in range(ET):
                    w_t = wp.tile([128, WCOLS], F32, tag="wt")
                    nc.sync.dma_start(
                        out=w_t[:],
                        in_=wt[et * 128 : (et + 1) * 128, g * WCOLS : (g + 1) * WCOLS],
                    )
                    wts.append(w_t)
                psums = [
                    ps.tile([BS, 512], F32, tag="psum", name=f"psum{c}")
                    for c in range(CHUNKS)
                ]
                # et-outer: stationary b_meanT[et] reused across CHUNKS matmuls
                for et in range(ET):
                    for c in range(CHUNKS):
                        nc.tensor.matmul(
                            out=psums[c][:],
                            lhsT=bmt[:, et, :],
                            rhs=wts[et][:, c * 512 : (c + 1) * 512],
                            start=(et == 0),
                            stop=(et == ET - 1),
                        )
                stage = tsp.tile([BS, WCOLS], F32)
                for c in range(CHUNKS):
                    nc.vector.tensor_copy(stage[:, c * 512 : (c + 1) * 512], psums[c][:])
                nc.scalar.dma_start(
                    out=t_out[:, g * WCOLS : (g + 1) * WCOLS], in_=stage[:]
                )
    nc.finalize()
    return nc


def _build_k3(apply_affine):
    nc = bacc.Bacc(trn_type="TRN2", num_devices=N_CORES)
    tb = nc.dram_tensor("tb", [B_SH, H, H], F32, kind="ExternalInput")  # [b, d, k]
    fatb = nc.dram_tensor("fatb", [B_SH, H, LEN_A], F32, kind="ExternalInput")
    fab = nc.dram_tensor("fab", [B_SH, LEN_A, H], F32, kind="ExternalInput")
    bias_d = nc.dram_tensor("bias", [H], F32, kind="ExternalInput")
    gamma_d = nc.dram_tensor("gamma", [H], F32, kind="ExternalInput")
    beta_d = nc.dram_tensor("beta", [H], F32, kind="ExternalInput")
    out = nc.dram_tensor("out", [B_SH, LEN_A, H], F32, kind="ExternalOutput")

    with tile.TileContext(nc) as tc:
        with (
            tc.tile_pool(name="consts", bufs=1) as consts,
            tc.tile_pool(name="ins", bufs=3) as ins,
            tc.tile_pool(name="ps", bufs=4, space="PSUM") as ps,
            tc.tile_pool(name="work", bufs=4) as work,
            tc.tile_pool(name="small", bufs=8) as small,
        ):
            gamma_t = consts.tile([128, H], F32)
            nc.sync.dma_start(out=gamma_t[:], in_=gamma_d.ap().partition_broadcast(128))
            beta_t = consts.tile([128, H], F32)
            nc.sync.dma_start(out=beta_t[:], in_=beta_d.ap().partition_broadcast(128))
            eps_t = consts.tile([128, 1], F32)
            nc.vector.memset(eps_t[:], LN_EPS)
            bias_t = consts.tile([128, H], F32)
            nc.sync.dma_start(out=bias_t[:], in_=bias_d.ap().partition_broadcast(128))

            for b in range(B_SH):
                # per-dt loads so the first matmul starts after 256 KB, not 1.7 MB
                t_t = ins.tile([128, ET, H], F32, tag="t")
                fat_t = ins.tile([128, ET, LEN_A], F32, tag="fat")
                for dt_i in range(ET):
                    nc.sync.dma_start(
                        out=fat_t[:, dt_i, :], in_=fatb[b, dt_i * 128 : (dt_i + 1) * 128, :]
                    )
                    nc.sync.dma_start(
                        out=t_t[:, dt_i, :], in_=tb[b, dt_i * 128 : (dt_i + 1) * 128, :]
                    )
                for a0, aw in A_TILES:
                    psum = ps.tile([aw, H], F32, tag="psum")
                    for dt_i in range(ET):
                        nc.tensor.matmul(
                            out=psum[:],
                            lhsT=fat_t[:, dt_i, a0 : a0 + aw],
                            rhs=t_t[:, dt_i, :],
                            start=(dt_i == 0),
                            stop=(dt_i == ET - 1),
                        )
                    fa_t = work.tile([aw, H], F32, tag="fa")
                    nc.sync.dma_start(out=fa_t[:], in_=fab[b, a0 : a0 + aw, :])
                    x = work.tile([aw, H], F32, tag="x")
                    nc.vector.tensor_add(out=x[:], in0=psum[:], in1=fa_t[:])
                    nc.vector.tensor_add(out=x[:], in0=x[:], in1=bias_t[:aw, :])
                    stats = small.tile([aw, 6], F32, tag="stats")
                    nc.vector.bn_stats(out=stats[:], in_=x[:])
                    mv = small.tile([aw, 2], F32, tag="mv")
                    nc.vector.bn_aggr(out=mv[:], in_=stats[:])
                    rstd = small.tile([aw, 1], F32, tag="rstd")
                    nc.scalar.activation(
                        out=rstd[:],
                        in_=mv[:, 1:2],
                        func=mybir.ActivationFunctionType.Sqrt,
                        bias=eps_t[:aw, :],
                        scale=1.0,
                    )
                    nc.vector.reciprocal(out=rstd[:], in_=rstd[:])
                    xn = work.tile([aw, H], F32, tag="xn")
                    nc.vector.tensor_scalar(
                        out=xn[:],
                        in0=x[:],
                        scalar1=mv[:, 0:1],
                        scalar2=rstd[:],
                        op0=mybir.AluOpType.subtract,
                        op1=mybir.AluOpType.mult,
                    )
                    if apply_affine:
                        nc.vector.tensor_mul(out=xn[:], in0=xn[:], in1=gamma_t[:aw, :])
                        nc.vector.tensor_add(out=xn[:], in0=xn[:], in1=beta_t[:aw, :])
                    nc.scalar.dma_start(out=out[b, a0 : a0 + aw, :], in_=xn[:])
    nc.finalize()
    return nc


_CACHE = {}


def _program(name, builder):
    if name not in _CACHE:
        _CACHE[name] = builder()
    return _CACHE[name]


def kernel(feat_a, feat_b, W, bias, gamma, beta, _trace=False, _timings=None):
    feat_a = np.ascontiguousarray(feat_a, dtype=np.float32)
    feat_b = np.ascontiguousarray(feat_b, dtype=np.float32)
    W = np.ascontiguousarray(W, dtype=np.float32)
    bias = np.ascontiguousarray(bias, dtype=np.float32)
    gamma = np.ascontiguousarray(gamma, dtype=np.float32)
    beta = np.ascontiguousarray(beta, dtype=np.float32)

    core_ids = list(range(N_CORES))
    affine = not (np.all(gamma == 1.0) and np.all(beta == 0.0))
    nc1 = _program("k1", _build_k1)
    nc2 = _program("k2", _build_k2)
    nc3 = _program(("k3", affine), lambda: _build_k3(affine))
    trace_kw = dict(trace=True, trace_cores=[0]) if _trace else {}

    # ---- K1: partial b_mean over j-shards ----
    in_maps1 = [
        {
            "fbt": np.ascontiguousarray(
                feat_b[:, i * J_SH : (i + 1) * J_SH, :].transpose(2, 0, 1)
            )
        }
        for i in range(N_CORES)
    ]
    res1 = run_bass_kernel_spmd(nc1, in_maps1, core_ids, **trace_kw)
    if _timings is not None:
        _timings.append(res1.exec_time_ns)
    bmT = np.sum([res1.results[i]["pb"] for i in range(N_CORES)], axis=0)
    bmT = np.ascontiguousarray(bmT, dtype=np.float32)

    # ---- K2: t = W x b_mean, k-sharded W stream ----
    in_maps2 = []
    for i in range(N_CORES):
        wi = np.ascontiguousarray(
            W[i * K_SH : (i + 1) * K_SH].transpose(2, 1, 0)
        ).reshape(H, DK)
        in_maps2.append({"bm": bmT, "wt": wi})
    res2 = run_bass_kernel_spmd(nc2, in_maps2, core_ids, **trace_kw)
    if _timings is not None:
        _timings.append(res2.exec_time_ns)
    t_full = np.concatenate(
        [res2.results[i]["t_out"].reshape(BS, H, K_SH) for i in range(N_CORES)], axis=2
    )

    # ---- K3: fused matmul + residual + LayerNorm, batch-sharded ----
    in_maps3 = []
    for j in range(N_CORES):
        bsl = slice(j * B_SH, (j + 1) * B_SH)
        in_maps3.append(
            {
                "tb": np.ascontiguousarray(t_full[bsl]),
                "fatb": np.ascontiguousarray(feat_a[bsl].transpose(0, 2, 1)),
                "fab": np.ascontiguousarray(feat_a[bsl]),
                "bias": bias,
                "gamma": gamma,
                "beta": beta,
            }
        )
    res3 = run_bass_kernel_spmd(nc3, in_maps3, core_ids, **trace_kw)
    if _timings is not None:
        _timings.append(res3.exec_time_ns)

    return np.concatenate([res3.results[j]["out"] for j in range(N_CORES)], axis=0)


# revision 20
# speedup vs baseline: 1.2153x; 1.0009x over previous
"""Trainium2 Bass kernel for nn_BilinearAttentionFusion.

Math (see reference):
    b_mean = mean_j feat_b[b, j, :]                      [32, 512]
    t[b, k, d] = sum_e W[k, d, e] * b_mean[b, e]         [32, 512, 512]
    fused = feat_a @ t^T + bias                          [32, 300, 512]
    out = LayerNorm(fused + feat_a) * gamma + beta

Distribution (8 NeuronCores, 3 SPMD launches, no collectives —
collectives cost 60-170us of cross-core sync under this runtime):
    K1 (j-sharded): core i reduces feat_b[:, 128i:128(i+1), :] to a
        partial b_meanT [e, b] (scaled 1/1024). Host sums the 8 partials.
    K2 (k-sharded): core i owns W[64i:64(i+1)] (64 MB fp32),
        host-transposed to [e, (d, k_loc)]. Streams it through the PE as
        the moving operand vs the tiny stationary b_meanT
        -> t_shard [32, (d, k_loc)]. Pure per-core streaming, no sync.
    host: concat t shards over k -> t[b, d, k], reshard by batch.
    K3 (batch-sharded): core j owns batches 4j..4j+3:
        fused[b] = feat_aT[b]^T @ t[b] (contract d), + bias + residual,
        LayerNorm, gamma/beta (skipped when exactly ones/zeros).

All matmuls, reductions and accumulations are full fp32 (the fp32 PE
moving-operand cost of 4 cyc/row overlaps the HBM-bound W stream).
"""
import sys

for _p in ("/opt/trn_rl_repo", "/root/.axon_site", "/root/.axon_site/_ro/pypackages"):
    if _p not in sys.path:
        sys.path.append(_p)

import numpy as np
import concourse.bacc as bacc
import concourse.tile as tile
from concourse import mybir
from concourse.bass_utils import run_bass_kernel_spmd

N_CORES = 8
BS, LEN_A, LEN_B, H = 32, 300, 1024, 512
K_SH = H // N_CORES  # 64 k-columns of W per core in K2
B_SH = BS // N_CORES  # 4 batches per core in K3
J_SH = LEN_B // N_CORES  # 128 j-rows of feat_b per core in K1
LN_EPS = 1e-5

F32 = mybir.dt.float32
F32R = mybir.dt.float32r  # PE-native reduced fp32: full-rate stream vs 4 cyc/row

WCOLS = 2048  # K2 W-streaming tile free size (1 MiB tiles)
DK = H * K_SH  # 32768 flattened (d, k_loc) columns per core
N_GROUPS = DK // WCOLS  # 16
CHUNKS = WCOLS // 512  # 4 psum chunks per group
ET = H // 128  # 4 contraction e-tiles
A_TILES = [(0, 128), (128, 128), (256, 44)]  # len_a = 300


def _build_k1():
    nc = bacc.Bacc(trn_type="TRN2", num_devices=N_CORES)
    fbt = nc.dram_tensor("fbt", [H, BS, J_SH], F32, kind="ExternalInput")
    pb_out = nc.dram_tensor("pb", [H, BS], F32, kind="ExternalOutput")
    with tile.TileContext(nc) as tc:
        with (
            tc.tile_pool(name="fb", bufs=3) as fbp,
            tc.tile_pool(name="small", bufs=4) as small,
        ):
            # finer b-halves pipeline DMA with the DVE reduce
            for et in range(ET):
                pb = small.tile([128, BS], F32)
                for h in range(2):
                    bs0 = h * (BS // 2)
                    fb_t = fbp.tile([128, BS // 2, J_SH], F32, tag="fb")
                    nc.sync.dma_start(
                        out=fb_t[:],
                        in_=fbt[et * 128 : (et + 1) * 128, bs0 : bs0 + BS // 2, :],
                    )
                    nc.vector.reduce_sum(
                        out=pb[:, bs0 : bs0 + BS // 2],
                        in_=fb_t[:],
                        axis=mybir.AxisListType.X,
                    )
                nc.scalar.mul(out=pb[:], in_=pb[:], mul=1.0 / LEN_B)
                nc.scalar.dma_start(out=pb_out[et * 128 : (et + 1) * 128, :], in_=pb[:])
    nc.finalize()
    return nc


def _build_k2():
    nc = bacc.Bacc(trn_type="TRN2", num_devices=N_CORES)
    bm = nc.dram_tensor("bm", [H, BS], F32, kind="ExternalInput")
    wt = nc.dram_tensor("wt", [H, DK], F32, kind="ExternalInput")
    # chunk-major layout so the 256 KB stage writes use all 128 partitions
    t_out = nc.dram_tensor("t_out", [DK // 512, BS, 512], F32, kind="ExternalOutput")

    with tile.TileContext(nc) as tc:
        with (
            tc.tile_pool(name="bm", bufs=1) as bmp,
            tc.tile_pool(name="wtiles", bufs=12) as wp,
            tc.tile_pool(name="ps", bufs=8, space="PSUM") as ps,
            tc.tile_pool(name="tstage", bufs=3) as tsp,
        ):
            bmt = bmp.tile([128, ET, BS], F32)
            nc.sync.dma_start(out=bmt[:], in_=bm.ap().rearrange("(t p) b -> p t b", p=128))

            for g in range(N_GROUPS):
                wts = []
                for et in range(ET):
                    w_t = wp.tile([128, WCOLS], F32, tag="wt")
                    nc.sync.dma_start(
                        out=w_t[:],
                        in_=wt[et * 128 : (et + 1) * 128, g * WCOLS : (g + 1) * WCOLS],
                    )
                    wts.append(w_t)
                psums = [
                    ps.tile([BS, 512], F32, tag="psum", name=f"psum{c}")
                    for c in range(CHUNKS)
                ]
                for et in range(ET):
                    for c in range(CHUNKS):
                        nc.tensor.matmul(
                            out=psums[c][:],
                            lhsT=bmt[:, et, :],
                            rhs=wts[et][:, c * 512 : (c + 1) * 512],
                            start=(et == 0),
                            stop=(et == ET - 1),
                        )
                for c in range(CHUNKS):
                    stage = tsp.tile([BS, 512], F32, tag="stage", name=f"st{c}")
                    nc.vector.tensor_copy(stage[:], psums[c][:])
                    nc.scalar.dma_start(out=t_out[g * CHUNKS + c, :, :], in_=stage[:])
    nc.finalize()
    return nc


def _build_k3(apply_affine):
    nc = bacc.Bacc(trn_type="TRN2", num_devices=N_CORES)
    tb = nc.dram_tensor("tb", [B_SH, H, H], F32, kind="ExternalInput")  # [b, d, k]
    fatb = nc.dram_tensor("fatb", [B_SH, H, LEN_A], F32, kind="ExternalInput")
    fab = nc.dram_tensor("fab", [B_SH, LEN_A, H], F32, kind="ExternalInput")
    bias_d = nc.dram_tensor("bias", [H], F32, kind="ExternalInput")
    gamma_d = nc.dram_tensor("gamma", [H], F32, kind="ExternalInput")
    beta_d = nc.dram_tensor("beta", [H], F32, kind="ExternalInput")
    out = nc.dram_tensor("out", [B_SH, LEN_A, H], F32, kind="ExternalOutput")

    with tile.TileContext(nc) as tc:
        with (
            tc.tile_pool(name="consts", bufs=1) as consts,
            tc.tile_pool(name="ins", bufs=3) as ins,
            tc.tile_pool(name="ps", bufs=4, space="PSUM") as ps,
            tc.tile_pool(name="work", bufs=4) as work,
            tc.tile_pool(name="small", bufs=8) as small,
        ):
            gamma_t = consts.tile([128, H], F32)
            nc.sync.dma_start(out=gamma_t[:], in_=gamma_d.ap().partition_broadcast(128))
            beta_t = consts.tile([128, H], F32)
            nc.sync.dma_start(out=beta_t[:], in_=beta_d.ap().partition_broadcast(128))
            eps_t = consts.tile([128, 1], F32)
            nc.vector.memset(eps_t[:], LN_EPS)
            bias_t = consts.tile([128, H], F32)
            nc.sync.dma_start(out=bias_t[:], in_=bias_d.ap().partition_broadcast(128))

            for b in range(B_SH):
                # per-dt loads so the first matmul starts after 256 KB, not 1.7 MB
                t_t = ins.tile([128, ET, H], F32, tag="t")
                fat_t = ins.tile([128, ET, LEN_A], F32, tag="fat")
                for dt_i in range(ET):
                    nc.sync.dma_start(
                        out=fat_t[:, dt_i, :], in_=fatb[b, dt_i * 128 : (dt_i + 1) * 128, :]
                    )
                    nc.sync.dma_start(
                        out=t_t[:, dt_i, :], in_=tb[b, dt_i * 128 : (dt_i + 1) * 128, :]
                    )
                for a0, aw in A_TILES:
                    psum = ps.tile([aw, H], F32, tag="psum")
                    for dt_i in range(ET):
                        nc.tensor.matmul(
                            out=psum[:],
                            lhsT=fat_t[:, dt_i, a0 : a0 + aw],
                            rhs=t_t[:, dt_i, :],
                            start=(dt_i == 0),
                            stop=(dt_i == ET - 1),
                        )
                    fa_t = work.tile([aw, H], F32, tag="fa")
                    nc.sync.dma_start(out=fa_t[:], in_=fab[b, a0 : a0 + aw, :])
                    x = work.tile([aw, H], F32, tag="x")
                    nc.vector.tensor_add(out=x[:], in0=psum[:], in1=fa_t[:])
                    nc.vector.tensor_add(out=x[:], in0=x[:], in1=bias_t[:aw, :])
                    stats = small.tile([aw, 6], F32, tag="stats")
                    nc.vector.bn_stats(out=stats[:], in_=x[:])
                    mv = small.tile([aw, 2], F32, tag="mv")
                    nc.vector.bn_aggr(out=mv[:], in_=stats[:])
                    rstd = small.tile([aw, 1], F32, tag="rstd")
                    nc.scalar.activation(
                        out=rstd[:],
                        in_=mv[:, 1:2],
                        func=mybir.ActivationFunctionType.Sqrt,
                        bias=eps_t[:aw, :],
                        scale=1.0,
                    )
                    nc.vector.reciprocal(out=rstd[:], in_=rstd[:])
                    xn = work.tile([aw, H], F32, tag="xn")
                    nc.vector.tensor_scalar(
                        out=xn[:],
                        in0=x[:],
                        scalar1=mv[:, 0:1],
                        scalar2=rstd[:],
                        op0=mybir.AluOpType.subtract,
                        op1=mybir.AluOpType.mult,
                    )
                    if apply_affine:
                        nc.vector.tensor_mul(out=xn[:], in0=xn[:], in1=gamma_t[:aw, :])
                        nc.vector.tensor_add(out=xn[:], in0=xn[:], in1=beta_t[:aw, :])
                    nc.scalar.dma_start(out=out[b, a0 : a0 + aw, :], in_=xn[:])
    nc.finalize()
    return nc


_CACHE = {}


def _program(name, builder):
    if name not in _CACHE:
        _CACHE[name] = builder()
    return _CACHE[name]


def kernel(feat_a, feat_b, W, bias, gamma, beta, _trace=False, _timings=None):
    feat_a = np.ascontiguousarray(feat_a, dtype=np.float32)
    feat_b = np.ascontiguousarray(feat_b, dtype=np.float32)
    W = np.ascontiguousarray(W, dtype=np.float32)
    bias = np.ascontiguousarray(bias, dtype=np.float32)
    gamma = np.ascontiguousarray(gamma, dtype=np.float32)
    beta = np.ascontiguousarray(beta, dtype=np.float32)

    core_ids = list(range(N_CORES))
    affine = not (np.all(gamma == 1.0) and np.all(beta == 0.0))
    nc1 = _program("k1", _build_k1)
    nc2 = _program("k2", _build_k2)
    nc3 = _program(("k3", affine), lambda: _build_k3(affine))
    trace_kw = dict(trace=True, trace_cores=[0]) if _trace else {}

    # ---- K1: partial b_mean over j-shards ----
    in_maps1 = [
        {
            "fbt": np.ascontiguousarray(
                feat_b[:, i * J_SH : (i + 1) * J_SH, :].transpose(2, 0, 1)
            )
        }
        for i in range(N_CORES)
    ]
    res1 = run_bass_kernel_spmd(nc1, in_maps1, core_ids, **trace_kw)
    if _timings is not None:
        _timings.append(res1.exec_time_ns)
    bmT = np.sum([res1.results[i]["pb"] for i in range(N_CORES)], axis=0)
    bmT = np.ascontiguousarray(bmT, dtype=np.float32)

    # ---- K2: t = W x b_mean, k-sharded W stream ----
    in_maps2 = []
    for i in range(N_CORES):
        wi = np.ascontiguousarray(
            W[i * K_SH : (i + 1) * K_SH].transpose(2, 1, 0)
        ).reshape(H, DK)
        in_maps2.append({"bm": bmT, "wt": wi})
    res2 = run_bass_kernel_spmd(nc2, in_maps2, core_ids, **trace_kw)
    if _timings is not None:
        _timings.append(res2.exec_time_ns)
    t_full = np.concatenate(
        [
            # [chunk, b, 512] -> [b, chunk*512 = (d, k_loc)] -> [b, d, k_loc]
            res2.results[i]["t_out"].transpose(1, 0, 2).reshape(BS, H, K_SH)
            for i in range(N_CORES)
        ],
        axis=2,
    )

    # ---- K3: fused matmul + residual + LayerNorm, batch-sharded ----
    in_maps3 = []
    for j in range(N_CORES):
        bsl = slice(j * B_SH, (j + 1) * B_SH)
        in_maps3.append(
            {
                "tb": np.ascontiguousarray(t_full[bsl]),
                "fatb": np.ascontiguousarray(feat_a[bsl].transpose(0, 2, 1)),
                "fab": np.ascontiguousarray(feat_a[bsl]),
                "bias": bias,
                "gamma": gamma,
                "beta": beta,
            }
        )
    res3 = run_bass_kernel_spmd(nc3, in_maps3, core_ids, **trace_kw)
    if _timings is not None:
        _timings.append(res3.exec_time_ns)

    return np.concatenate([res3.results[j]["out"] for j in range(N_CORES)], axis=0)


# revision 22
# speedup vs baseline: 1.2244x; 1.0074x over previous
"""Trainium2 Bass kernel for nn_BilinearAttentionFusion.

Math (see reference):
    b_mean = mean_j feat_b[b, j, :]                      [32, 512]
    t[b, k, d] = sum_e W[k, d, e] * b_mean[b, e]         [32, 512, 512]
    fused = feat_a @ t^T + bias                          [32, 300, 512]
    out = LayerNorm(fused + feat_a) * gamma + beta

Distribution (8 NeuronCores, 3 SPMD launches, no collectives —
collectives cost 60-170us of cross-core sync under this runtime):
    K1 (j-sharded): core i reduces feat_b[:, 128i:128(i+1), :] to a
        partial b_meanT [e, b] (scaled 1/1024). Host sums the 8 partials.
    K2 (k-sharded): core i owns W[64i:64(i+1)] (64 MB fp32),
        host-transposed to [e, (d, k_loc)]. Streams it through the PE as
        the moving operand vs the tiny stationary b_meanT
        -> t_shard [32, (d, k_loc)]. Pure per-core streaming, no sync.
    host: concat t shards over k -> t[b, d, k], reshard by batch.
    K3 (batch-sharded): core j owns batches 4j..4j+3:
        fused[b] = feat_aT[b]^T @ t[b] (contract d), + bias + residual,
        LayerNorm, gamma/beta (skipped when exactly ones/zeros).

All matmuls, reductions and accumulations are full fp32 (the fp32 PE
moving-operand cost of 4 cyc/row overlaps the HBM-bound W stream).
"""
import sys

for _p in ("/opt/trn_rl_repo", "/root/.axon_site", "/root/.axon_site/_ro/pypackages"):
    if _p not in sys.path:
        sys.path.append(_p)

import numpy as np
import concourse.bacc as bacc
import concourse.tile as tile
from concourse import mybir
from concourse.bass_utils import run_bass_kernel_spmd

N_CORES = 8
BS, LEN_A, LEN_B, H = 32, 300, 1024, 512
K_SH = H // N_CORES  # 64 k-columns of W per core in K2
B_SH = BS // N_CORES  # 4 batches per core in K3
J_SH = LEN_B // N_CORES  # 128 j-rows of feat_b per core in K1
LN_EPS = 1e-5

F32 = mybir.dt.float32
F32R = mybir.dt.float32r  # PE-native reduced fp32: full-rate stream vs 4 cyc/row

WCOLS = 2048  # K2 W-streaming tile free size (1 MiB tiles)
DK = H * K_SH  # 32768 flattened (d, k_loc) columns per core
N_GROUPS = DK // WCOLS  # 16
CHUNKS = WCOLS // 512  # 4 psum chunks per group
ET = H // 128  # 4 contraction e-tiles
A_TILES = [(0, 128), (128, 128), (256, 44)]  # len_a = 300


def _build_k1():
    nc = bacc.Bacc(trn_type="TRN2", num_devices=N_CORES)
    fbt = nc.dram_tensor("fbt", [H, BS, J_SH], F32, kind="ExternalInput")
    pb_out = nc.dram_tensor("pb", [H, BS], F32, kind="ExternalOutput")
    with tile.TileContext(nc) as tc:
        with (
            tc.tile_pool(name="fb", bufs=3) as fbp,
            tc.tile_pool(name="small", bufs=4) as small,
        ):
            # finer b-halves pipeline DMA with the DVE reduce
            for et in range(ET):
                pb = small.tile([128, BS], F32)
                for h in range(2):
                    bs0 = h * (BS // 2)
                    fb_t = fbp.tile([128, BS // 2, J_SH], F32, tag="fb")
                    nc.sync.dma_start(
                        out=fb_t[:],
                        in_=fbt[et * 128 : (et + 1) * 128, bs0 : bs0 + BS // 2, :],
                    )
                    nc.vector.reduce_sum(
                        out=pb[:, bs0 : bs0 + BS // 2],
                        in_=fb_t[:],
                        axis=mybir.AxisListType.X,
                    )
                nc.scalar.mul(out=pb[:], in_=pb[:], mul=1.0 / LEN_B)
                nc.scalar.dma_start(out=pb_out[et * 128 : (et + 1) * 128, :], in_=pb[:])
    nc.finalize()
    return nc


def _build_k2():
    nc = bacc.Bacc(trn_type="TRN2", num_devices=N_CORES)
    bm = nc.dram_tensor("bm", [H, BS], F32, kind="ExternalInput")
    wt = nc.dram_tensor("wt", [H, DK], F32, kind="ExternalInput")
    # chunk-major layout so the 256 KB stage writes use all 128 partitions
    t_out = nc.dram_tensor("t_out", [DK // 512, BS, 512], F32, kind="ExternalOutput")

    with tile.TileContext(nc) as tc:
        with (
            tc.tile_pool(name="bm", bufs=1) as bmp,
            tc.tile_pool(name="wtiles", bufs=14) as wp,
            tc.tile_pool(name="ps", bufs=8, space="PSUM") as ps,
            tc.tile_pool(name="tstage", bufs=3) as tsp,
        ):
            bmt = bmp.tile([128, ET, BS], F32)
            nc.sync.dma_start(out=bmt[:], in_=bm.ap().rearrange("(t p) b -> p t b", p=128))

            # taper the final groups so the trailing PE work after the last
            # W DMA (which nothing overlaps) is half a group, not a full one
            groups = [(gi * WCOLS, WCOLS) for gi in range(N_GROUPS - 1)]
            groups += [(DK - WCOLS, WCOLS // 2), (DK - WCOLS // 2, WCOLS // 2)]
            for col0, width in groups:
                nchunk = width // 512
                wts = []
                for et in range(ET):
                    w_t = wp.tile([128, WCOLS], F32, tag="wt")
                    nc.sync.dma_start(
                        out=w_t[:, :width],
                        in_=wt[et * 128 : (et + 1) * 128, col0 : col0 + width],
                    )
                    wts.append(w_t)
                psums = [
                    ps.tile([BS, 512], F32, tag="psum", name=f"psum{c}")
                    for c in range(nchunk)
                ]
                for et in range(ET):
                    for c in range(nchunk):
                        nc.tensor.matmul(
                            out=psums[c][:],
                            lhsT=bmt[:, et, :],
                            rhs=wts[et][:, c * 512 : (c + 1) * 512],
                            start=(et == 0),
                            stop=(et == ET - 1),
                        )
                for c in range(nchunk):
                    stage = tsp.tile([BS, 512], F32, tag="stage", name=f"st{c}")
                    nc.vector.tensor_copy(stage[:], psums[c][:])
                    nc.scalar.dma_start(
                        out=t_out[col0 // 512 + c, :, :], in_=stage[:]
                    )
    nc.finalize()
    return nc


def _build_k3(apply_affine):
    nc = bacc.Bacc(trn_type="TRN2", num_devices=N_CORES)
    tb = nc.dram_tensor("tb", [B_SH, H, H], F32, kind="ExternalInput")  # [b, d, k]
    fatb = nc.dram_tensor("fatb", [B_SH, H, LEN_A], F32, kind="ExternalInput")
    fab = nc.dram_tensor("fab", [B_SH, LEN_A, H], F32, kind="ExternalInput")
    bias_d = nc.dram_tensor("bias", [H], F32, kind="ExternalInput")
    gamma_d = nc.dram_tensor("gamma", [H], F32, kind="ExternalInput")
    beta_d = nc.dram_tensor("beta", [H], F32, kind="ExternalInput")
    out = nc.dram_tensor("out", [B_SH, LEN_A, H], F32, kind="ExternalOutput")

    with tile.TileContext(nc) as tc:
        with (
            tc.tile_pool(name="consts", bufs=1) as consts,
            tc.tile_pool(name="ins", bufs=3) as ins,
            tc.tile_pool(name="ps", bufs=4, space="PSUM") as ps,
            tc.tile_pool(name="work", bufs=4) as work,
            tc.tile_pool(name="small", bufs=8) as small,
        ):
            gamma_t = consts.tile([128, H], F32)
            nc.sync.dma_start(out=gamma_t[:], in_=gamma_d.ap().partition_broadcast(128))
            beta_t = consts.tile([128, H], F32)
            nc.sync.dma_start(out=beta_t[:], in_=beta_d.ap().partition_broadcast(128))
            eps_t = consts.tile([128, 1], F32)
            nc.vector.memset(eps_t[:], LN_EPS)
            bias_t = consts.tile([128, H], F32)
            nc.sync.dma_start(out=bias_t[:], in_=bias_d.ap().partition_broadcast(128))

            for b in range(B_SH):
                # per-dt loads so the first matmul starts after 256 KB, not 1.7 MB
                t_t = ins.tile([128, ET, H], F32, tag="t")
                fat_t = ins.tile([128, ET, LEN_A], F32, tag="fat")
                for dt_i in range(ET):
                    nc.sync.dma_start(
                        out=fat_t[:, dt_i, :], in_=fatb[b, dt_i * 128 : (dt_i + 1) * 128, :]
                    )
                    nc.sync.dma_start(
                        out=t_t[:, dt_i, :], in_=tb[b, dt_i * 128 : (dt_i + 1) * 128, :]
                    )
                for a0, aw in A_TILES:
                    psum = ps.tile([aw, H], F32, tag="psum")
                    for dt_i in range(ET):
                        nc.tensor.matmul(
                            out=psum[:],
                            lhsT=fat_t[:, dt_i, a0 : a0 + aw],
                            rhs=t_t[:, dt_i, :],
                            start=(dt_i == 0),
                            stop=(dt_i == ET - 1),
                        )
                    fa_t = work.tile([aw, H], F32, tag="fa")
                    nc.sync.dma_start(out=fa_t[:], in_=fab[b, a0 : a0 + aw, :])
                    x = work.tile([aw, H], F32, tag="x")
                    nc.vector.tensor_add(out=x[:], in0=psum[:], in1=fa_t[:])
                    nc.vector.tensor_add(out=x[:], in0=x[:], in1=bias_t[:aw, :])
                    stats = small.tile([aw, 6], F32, tag="stats")
                    nc.vector.bn_stats(out=stats[:], in_=x[:])
                    mv = small.tile([aw, 2], F32, tag="mv")
                    nc.vector.bn_aggr(out=mv[:], in_=stats[:])
                    rstd = small.tile([aw, 1], F32, tag="rstd")
                    nc.scalar.activation(
                        out=rstd[:],
                        in_=mv[:, 1:2],
                        func=mybir.ActivationFunctionType.Sqrt,
                        bias=eps_t[:aw, :],
                        scale=1.0,
                    )
                    nc.vector.reciprocal(out=rstd[:], in_=rstd[:])
                    xn = work.tile([aw, H], F32, tag="xn")
                    nc.vector.tensor_scalar(
                        out=xn[:],
                        in0=x[:],
                        scalar1=mv[:, 0:1],
                        scalar2=rstd[:],
                        op0=mybir.AluOpType.subtract,
                        op1=mybir.AluOpType.mult,
                    )
                    if apply_affine:
                        nc.vector.tensor_mul(out=xn[:], in0=xn[:], in1=gamma_t[:aw, :])
                        nc.vector.tensor_add(out=xn[:], in0=xn[:], in1=beta_t[:aw, :])
                    nc.scalar.dma_start(out=out[b, a0 : a0 + aw, :], in_=xn[:])
    nc.finalize()
    return nc


_CACHE = {}


def _program(name, builder):
    if name not in _CACHE:
        _CACHE[name] = builder()
    return _CACHE[name]


def kernel(feat_a, feat_b, W, bias, gamma, beta, _trace=False, _timings=None):
    feat_a = np.ascontiguousarray(feat_a, dtype=np.float32)
    feat_b = np.ascontiguousarray(feat_b, dtype=np.float32)
    W = np.ascontiguousarray(W, dtype=np.float32)
    bias = np.ascontiguousarray(bias, dtype=np.float32)
    gamma = np.ascontiguousarray(gamma, dtype=np.float32)
    beta = np.ascontiguousarray(beta, dtype=np.float32)

    core_ids = list(range(N_CORES))
    affine = not (np.all(gamma == 1.0) and np.all(beta == 0.0))
    nc1 = _program("k1", _build_k1)
    nc2 = _program("k2", _build_k2)
    nc3 = _program(("k3", affine), lambda: _build_k3(affine))
    trace_kw = dict(trace=True, trace_cores=[0]) if _trace else {}

    # ---- K1: partial b_mean over j-shards ----
    in_maps1 = [
        {
            "fbt": np.ascontiguousarray(
                feat_b[:, i * J_SH : (i + 1) * J_SH, :].transpose(2, 0, 1)
            )
        }
        for i in range(N_CORES)
    ]
    res1 = run_bass_kernel_spmd(nc1, in_maps1, core_ids, **trace_kw)
    if _timings is not None:
        _timings.append(res1.exec_time_ns)
    bmT = np.sum([res1.results[i]["pb"] for i in range(N_CORES)], axis=0)
    bmT = np.ascontiguousarray(bmT, dtype=np.float32)

    # ---- K2: t = W x b_mean, k-sharded W stream ----
    in_maps2 = []
    for i in range(N_CORES):
        wi = np.ascontiguousarray(
            W[i * K_SH : (i + 1) * K_SH].transpose(2, 1, 0)
        ).reshape(H, DK)
        in_maps2.append({"bm": bmT, "wt": wi})
    res2 = run_bass_kernel_spmd(nc2, in_maps2, core_ids, **trace_kw)
    if _timings is not None:
        _timings.append(res2.exec_time_ns)
    t_full = np.concatenate(
        [
            # [chunk, b, 512] -> [b, chunk*512 = (d, k_loc)] -> [b, d, k_loc]
            res2.results[i]["t_out"].transpose(1, 0, 2).reshape(BS, H, K_SH)
            for i in range(N_CORES)
        ],
        axis=2,
    )

    # ---- K3: fused matmul + residual + LayerNorm, batch-sharded ----
    in_maps3 = []
    for j in range(N_CORES):
        bsl = slice(j * B_SH, (j + 1) * B_SH)
        in_maps3.append(
            {
                "tb": np.ascontiguousarray(t_full[bsl]),
                "fatb": np.ascontiguousarray(feat_a[bsl].transpose(0, 2, 1)),
                "fab": np.ascontiguousarray(feat_a[bsl]),
                "bias": bias,
                "gamma": gamma,
                "beta": beta,
            }
        )
    res3 = run_bass_kernel_spmd(nc3, in_maps3, core_ids, **trace_kw)
    if _timings is not None:
        _timings.append(res3.exec_time_ns)

    return np.concatenate([res3.results[j]["out"] for j in range(N_CORES)], axis=0)
